# revision 5
# baseline (speedup 1.0000x reference)
"""Trainium2 Bass kernel for nn_DebiasIntraDist (segment_reduce).

Full-input contract: kernel(**inputs) takes the complete (unsharded) inputs
and returns the full scalar loss. The N=65536 samples are sharded across the
8 NeuronCores by (demog, label-half): core 2d+h gets the rows with
demog == d and label-half h (a partition of the N axis). Every core then
owns a disjoint set of 256 (demog, label) groups, so no cross-core
reduction of group accumulators is needed at all.

Design (v3):
  * feats are quantized to fp16 on the host -> HBM traffic halves (the
    DMA floor is ~24 us/core). All on-device arithmetic on the quantized
    data is fp32-accumulated and near-exact, so the only meaningful error
    is the fp16 input rounding itself (~5e-4 relative on the loss).
  * within each core, rows are bucketed by group-chunk (local group id
    <128 vs >=128) so every 128-row tile touches a single 128-wide
    one-hot chunk -> ONE 512-wide matmul per tile instead of four.
  * per-tile work is spread across three otherwise-idle engines:
    one-hots alternate Vector/GpSimd, row sum-of-squares alternates
    Scalar (fused Square+accum activation) and Vector (fused
    scalar_tensor_tensor with accum_out).
  * group counts are sharding metadata; the host knows them exactly
    (bincount), so the device only accumulates sums[g,:] and sumsq[g].
  * no collective: each core DMAs out its 256 groups' statistics and the
    host does the final 2048-group scalar reduction (the v1 AllGather
    mesh cost ~35 us of serial tail).

Math per group: sums[g, :] (one-hot matmul) and sumsq[g] via the
variance decomposition
    sum_{i in g} ||x_i - mu_g||^2 = sumsq[g] - ||sums[g]||^2 / cnt[g].
The per-row ||x_i||^2 is centered by -D before its fp16 trip through the
small matmul (row norms are ~D +- sqrt(2D), so centering keeps the fp16
rounding ~15x smaller); the host adds D*cnt back.
"""

import numpy as np

try:
    import concourse.bacc as bacc
except ImportError:  # fresh environment without PYTHONPATH set up
    import sys
    for p in ("/root/.axon_site/_ro/trn_rl_repo", "/opt/trn_rl_repo",
              "/root/.axon_site/_ro/pypackages"):
        if p not in sys.path:
            sys.path.append(p)
    import concourse.bacc as bacc
import concourse.mybir as mybir
import concourse.tile as tile
import concourse.bass_utils as bass_utils

N_CORES = 8
P = 128
D = 512          # feature dim
NL = 256         # labels per core after (demog, label-half) sharding
ND = 4           # demog values
CH = 6           # sample-tiles per feats DMA (768 KiB)
PAD_LAB = 300.0  # pad label; never matches iota 0..127

# engine split knobs (fractions of the T tiles)
SQ_ACT_FRAC = 0.39   # row-sumsq tiles on ScalarE (rest: VectorE STT)
OH_GPS_FRAC = 0.50   # one-hot tiles on GpSimd (rest: VectorE)

_cache: dict[tuple, object] = {}


def _spread(T, frac):
    n = round(T * frac)
    return [(t * n) // T != ((t + 1) * n) // T for t in range(T)]


def _build(T0: int, T1: int):
    """Compile the SPMD kernel: T0 tiles of chunk 0 then T1 of chunk 1."""
    T = T0 + T1
    fp32 = mybir.dt.float32
    fp16 = mybir.dt.float16
    Alu = mybir.AluOpType
    Act = mybir.ActivationFunctionType

    sq_act = _spread(T, SQ_ACT_FRAC)   # True -> ScalarE does this row-sumsq
    oh_gps = _spread(T, OH_GPS_FRAC)   # True -> GpSimd builds this one-hot

    nc = bacc.Bacc("TRN2", target_bir_lowering=False, debug=False,
                   enable_asserts=True, num_devices=N_CORES)

    feats_t = nc.dram_tensor("feats_t", [P, T, D], fp16,
                             kind="ExternalInput").ap()
    labels_t = nc.dram_tensor("labels_t", [P, T], fp32,
                              kind="ExternalInput").ap()
    stats = nc.dram_tensor("stats", [P, 4], fp32, kind="ExternalOutput").ap()

    with tile.TileContext(nc) as tc:
        with (
            tc.tile_pool(name="const", bufs=1) as constp,
            tc.tile_pool(name="fx", bufs=8) as fxp,
            tc.tile_pool(name="oh", bufs=14) as ohp,
            tc.tile_pool(name="sq6", bufs=3) as sq6p,
            tc.tile_pool(name="r26", bufs=3) as r26p,
            tc.tile_pool(name="scrv", bufs=3) as scrvp,
            tc.tile_pool(name="scra", bufs=3) as scrap,
            tc.tile_pool(name="post", bufs=1) as postp,
            tc.tile_pool(name="ps", bufs=1, space="PSUM") as psp,
        ):
            # constants
            iota = constp.tile([P, P], fp16, tag="iota")
            nc.gpsimd.iota(iota[:], [[1, P]], channel_multiplier=0,
                           allow_small_or_imprecise_dtypes=True)
            labs = constp.tile([P, T], fp32, tag="labs")
            nc.scalar.dma_start(out=labs[:], in_=labels_t[:])
            # touch the ACT Square table early so its ~2.7us load overlaps
            # the first feats DMAs instead of stalling the first ACT tile
            warm = constp.tile([P, 1], fp32, tag="warm")
            nc.gpsimd.memset(warm[:], 0.0)
            nc.scalar.activation(warm[:], warm[:], Act.Square)

            # per-group accumulators; a PSUM accumulation group owns its
            # whole bank, so each gets one
            ps_sums = [psp.tile([P, D], fp32, tag=f"sums{c}", name=f"sums{c}")
                       for c in range(2)]
            ps_small = [psp.tile([P, 1], fp32, tag=f"small{c}",
                                 name=f"small{c}") for c in range(2)]

            t = 0
            first_chunk = True
            while t < T:
                # small first chunk so PE work starts ASAP
                L = 1 if first_chunk else min(CH, T - t)
                first_chunk = False
                fx = fxp.tile([P, CH, D], fp16, tag="fx")
                nc.sync.dma_start(out=fx[:, :L, :], in_=feats_t[:, t:t + L, :])
                sq6 = sq6p.tile([P, CH], fp32, tag="sq6")
                r26 = r26p.tile([P, CH], fp16, tag="r26")
                ohs = []
                for j in range(L):
                    ti = t + j
                    X = fx[:, j, :]
                    # one-hot of this tile's local labels (exact in fp16)
                    oh = ohp.tile([P, P], fp16, tag="oh")
                    ohs.append(oh)
                    oh_eng = nc.gpsimd if oh_gps[ti] else nc.vector
                    oh_eng.tensor_scalar(
                        out=oh[:], in0=iota[:], scalar1=labs[:, ti:ti + 1],
                        scalar2=None, op0=Alu.is_equal,
                    )
                    # row sumsq (fp32 accumulator; `out` is scratch)
                    if sq_act[ti]:
                        scr = scrap.tile([P, D], fp16, tag="scra")
                        nc.scalar.activation(scr[:], X, Act.Square,
                                             accum_out=sq6[:, j:j + 1])
                    else:
                        scr = scrvp.tile([P, D], fp16, tag="scrv")
                        nc.vector.scalar_tensor_tensor(
                            out=scr[:], in0=X, scalar=1.0, in1=X,
                            op0=Alu.mult, op1=Alu.mult,
                            accum_out=sq6[:, j:j + 1])

                    c = 0 if ti < T0 else 1
                    st = ti == 0 or ti == T0
                    sp = ti == T0 - 1 or ti == T - 1
                    nc.tensor.matmul(out=ps_sums[c][:], lhsT=oh[:], rhs=X,
                                     start=st, stop=sp)
                # one centered fp16 conversion for the whole slab, then the
                # small matmuls read per-tile columns of it
                nc.vector.tensor_scalar(
                    out=r26[:, :L], in0=sq6[:, :L], scalar1=-float(D),
                    scalar2=None, op0=Alu.add)
                for j in range(L):
                    ti = t + j
                    c = 0 if ti < T0 else 1
                    st = ti == 0 or ti == T0
                    sp = ti == T0 - 1 or ti == T - 1
                    nc.tensor.matmul(out=ps_small[c][:], lhsT=ohs[j][:],
                                     rhs=r26[:, j:j + 1], start=st, stop=sp)
                t += L

            # post: per-group stats out; the 2048-group finale is host work
            out_t = postp.tile([P, 4], fp32, tag="out_t")
            for c in range(2):
                scr_n = postp.tile([P, D], fp32, tag=f"scr_n{c}")
                nc.scalar.activation(scr_n[:], ps_sums[c][:], Act.Square,
                                     accum_out=out_t[:, c:c + 1])
                nc.vector.tensor_copy(out=out_t[:, 2 + c:3 + c],
                                      in_=ps_small[c][:])
            nc.sync.dma_start(out=stats, in_=out_t[:])

    nc.compile()
    return nc


def _shard(feats, labels, demog):
    """Partition rows by (demog, label-half) across cores, then bucket by
    group-chunk (local label < 128 vs >= 128) within each core.

    Any row order within a bucket works: the device one-hot (iota vs
    label mod 128) routes each row to its group slot. Also returns the
    exact per-(core, chunk, slot) counts -- sharding metadata the host
    keeps for the finale.
    """
    half = (labels >= NL).astype(np.int32)
    shard_id = demog * 2 + half
    local = labels % NL          # 0..255 within the core
    chunk = local // P           # 0 or 1
    slot = (local % P).astype(np.float32)

    cnt = np.zeros((N_CORES, 2, P), np.int64)
    np.add.at(cnt, (shard_id, chunk, local % P), 1)

    feats16 = feats.astype(np.float16)
    rows0, rows1 = [], []
    for s in range(N_CORES):
        m = shard_id == s
        rows0.append(np.flatnonzero(m & (chunk == 0)))
        rows1.append(np.flatnonzero(m & (chunk == 1)))
    T0 = max(1, max(-(-len(r) // P) for r in rows0))
    T1 = max(1, max(-(-len(r) // P) for r in rows1))
    T = T0 + T1

    in_maps = []
    for s in range(N_CORES):
        f = np.zeros((T * P, D), np.float16)
        lab = np.full(T * P, PAD_LAB, np.float32)
        r0, r1 = rows0[s], rows1[s]
        f[:len(r0)] = feats16[r0]
        lab[:len(r0)] = slot[r0]
        f[T0 * P:T0 * P + len(r1)] = feats16[r1]
        lab[T0 * P:T0 * P + len(r1)] = slot[r1]
        # tile t, partition p <-> row t*128+p; device reads [P, T, D]
        ft = np.ascontiguousarray(f.reshape(T, P, D).transpose(1, 0, 2))
        lt = np.ascontiguousarray(lab.reshape(T, P).T)
        in_maps.append({"feats_t": ft, "labels_t": lt})
    return T0, T1, in_maps, cnt


def _combine(stats_per_core, cnt):
    """Host finale over the 2048 groups (the 'gather/unshard' step)."""
    num = np.zeros(ND, np.float64)
    den = np.zeros(ND, np.float64)
    for s, st in enumerate(stats_per_core):
        st = st.astype(np.float64)
        d = s // 2
        for c in range(2):
            norm2 = st[:, c]
            cg = cnt[s, c].astype(np.float64)
            sumsq = st[:, 2 + c] + float(D) * cg
            safe = np.maximum(cg, 1.0)
            grp = (sumsq - norm2 / safe) / safe
            pres = cg > 0
            num[d] += grp[pres].sum()
            den[d] += pres.sum()
    intra = num / np.maximum(den, 1.0)
    return np.float32(np.mean(np.abs(intra - intra.mean())))


def kernel(feats, labels, demog_labels, _results_out=None):
    feats = np.ascontiguousarray(np.asarray(feats), dtype=np.float32)
    labels = np.asarray(labels).astype(np.int32)
    demog = np.asarray(demog_labels).astype(np.int32)
    assert feats.ndim == 2 and feats.shape[1] == D

    T0, T1, in_maps, cnt = _shard(feats, labels, demog)
    key = (T0, T1)
    nc = _cache.get(key)
    if nc is None:
        nc = _cache.setdefault(key, _build(T0, T1))
    res = None
    last_exc = None
    for attempt in range(3):
        try:
            res = bass_utils.run_bass_kernel_spmd(
                nc, in_maps, core_ids=list(range(N_CORES)))
            break
        except Exception as e:  # transient axon worker hangups
            last_exc = e
            import time
            time.sleep(10)
    if res is None:
        raise last_exc
    if _results_out is not None:
        _results_out.append(res)
    return _combine([res.results[s]["stats"] for s in range(N_CORES)], cnt)


# revision 6
# speedup vs baseline: 2.2900x; 2.2900x over previous
"""Trainium2 Bass kernel for nn_DebiasIntraDist (segment_reduce).

Full-input contract: kernel(**inputs) takes the complete (unsharded) inputs
and returns the full scalar loss. The N=65536 samples are sharded across the
8 NeuronCores by (demog, label-half): core 2d+h gets the rows with
demog == d and label-half h (a partition of the N axis). Every core then
owns a disjoint set of 256 (demog, label) groups, so no cross-core
reduction of group accumulators is needed at all.

Design (v5):
  * feats are quantized to fp16 on the host -> HBM traffic halves (the
    DMA floor is ~24 us/core). All on-device arithmetic on the quantized
    data is fp32-accumulated, so the only meaningful error is the fp16
    input rounding itself (~5e-4 relative on the loss).
  * within each core, rows are bucketed by group-chunk (local group id
    <128 vs >=128) so every 128-row tile touches a single 128-wide
    one-hot chunk -> each feature element is streamed through the PE
    exactly twice (sums and squares), not 4x as in the v1 baseline.
  * per-group sumsq uses a second PE matmul over Y = X*X instead of
    per-row fused reductions on DVE/ACT (those run at 1x, ~600-930 ns
    per tile; the elementwise square is 2x-mode on DVE). Y production is
    column-split between the Vector and Scalar engines so both stay
    under the DMA floor.
  * group counts are sharding metadata; the host knows them exactly
    (bincount), so the device only accumulates sums[g,:] and sumsq[g].
  * no collective: each core DMAs out its 256 groups' statistics and the
    host does the final 2048-group scalar reduction (the v1 AllGather
    mesh cost ~35 us of serial tail).

Math per group: sums[g, :] and sumsqvec[g, :] (one-hot matmuls), then
    sum_{i in g} ||x_i - mu_g||^2 = sum_d sumsqvec[g,d] - ||sums[g]||^2 / cnt[g].
"""

import numpy as np

try:
    import concourse.bacc as bacc
except ImportError:  # fresh environment without PYTHONPATH set up
    import sys
    for p in ("/root/.axon_site/_ro/trn_rl_repo", "/opt/trn_rl_repo",
              "/root/.axon_site/_ro/pypackages"):
        if p not in sys.path:
            sys.path.append(p)
    import concourse.bacc as bacc
import concourse.mybir as mybir
import concourse.tile as tile
import concourse.bass_utils as bass_utils

N_CORES = 8
P = 128
D = 512          # feature dim
NL = 256         # labels per core after (demog, label-half) sharding
ND = 4           # demog values
CH = 6           # sample-tiles per feats DMA (768 KiB)
PAD_LAB = 300.0  # pad label; never matches iota 0..127
Y_WV = 192       # columns of Y squared on VectorE (rest: ScalarE)

_cache: dict[tuple, object] = {}


def _build(T0: int, T1: int):
    """Compile the SPMD kernel: T0 tiles of chunk 0 then T1 of chunk 1."""
    T = T0 + T1
    fp32 = mybir.dt.float32
    fp16 = mybir.dt.float16
    Alu = mybir.AluOpType
    Act = mybir.ActivationFunctionType

    nc = bacc.Bacc("TRN2", target_bir_lowering=False, debug=False,
                   enable_asserts=True, num_devices=N_CORES)

    feats_t = nc.dram_tensor("feats_t", [P, T, D], fp16,
                             kind="ExternalInput").ap()
    labels_t = nc.dram_tensor("labels_t", [P, T], fp32,
                              kind="ExternalInput").ap()
    stats = nc.dram_tensor("stats", [P, 4], fp32, kind="ExternalOutput").ap()

    with tile.TileContext(nc) as tc:
        with (
            tc.tile_pool(name="const", bufs=1) as constp,
            tc.tile_pool(name="fx", bufs=8) as fxp,
            tc.tile_pool(name="oh", bufs=16) as ohp,
            tc.tile_pool(name="yy", bufs=3) as yyp,
            tc.tile_pool(name="post", bufs=1) as postp,
            tc.tile_pool(name="ps", bufs=1, space="PSUM") as psp,
        ):
            # constants
            iota = constp.tile([P, P], fp16, tag="iota")
            nc.gpsimd.iota(iota[:], [[1, P]], channel_multiplier=0,
                           allow_small_or_imprecise_dtypes=True)
            labs = constp.tile([P, T], fp32, tag="labs")
            nc.scalar.dma_start(out=labs[:], in_=labels_t[:])
            # touch the ACT Square table early so its ~2.7us load overlaps
            # the first feats DMAs instead of stalling the first ACT square
            warm = constp.tile([P, 1], fp32, tag="warm")
            nc.gpsimd.memset(warm[:], 0.0)
            nc.scalar.activation(warm[:], warm[:], Act.Square)

            # per-group accumulators; a PSUM accumulation group owns its
            # whole bank, so each gets one
            ps_sums = [psp.tile([P, D], fp32, tag=f"sums{c}", name=f"sums{c}")
                       for c in range(2)]
            ps_ssq = [psp.tile([P, D], fp32, tag=f"ssq{c}", name=f"ssq{c}")
                      for c in range(2)]
            out_t = postp.tile([P, 4], fp32, tag="out_t")

            def readout(c):
                # norm2[g] and sumsq[g] for chunk c; chunk 0's runs
                # mid-loop, overlapped with chunk 1's compute
                scr_n = postp.tile([P, D], fp32, tag=f"scr_n{c}")
                nc.scalar.activation(scr_n[:], ps_sums[c][:], Act.Square,
                                     accum_out=out_t[:, c:c + 1])
                nc.vector.tensor_reduce(
                    out=out_t[:, 2 + c:3 + c], in_=ps_ssq[c][:],
                    axis=mybir.AxisListType.X, op=Alu.add)

            t = 0
            first_chunk = True
            while t < T:
                # small first chunk so PE work starts ASAP
                L = 1 if first_chunk else min(CH, T - t)
                first_chunk = False
                fx = fxp.tile([P, CH, D], fp16, tag="fx")
                nc.sync.dma_start(out=fx[:, :L, :], in_=feats_t[:, t:t + L, :])
                ohs = []
                for j in range(L):
                    ti = t + j
                    # one-hot of this tile's local labels (exact in fp16)
                    oh = ohp.tile([P, P], fp16, tag="oh")
                    ohs.append(oh)
                    nc.vector.tensor_scalar(
                        out=oh[:], in0=iota[:], scalar1=labs[:, ti:ti + 1],
                        scalar2=None, op0=Alu.is_equal,
                    )
                    c = 0 if ti < T0 else 1
                    nc.tensor.matmul(out=ps_sums[c][:], lhsT=oh[:],
                                     rhs=fx[:, j, :],
                                     start=ti == 0 or ti == T0,
                                     stop=ti == T0 - 1 or ti == T - 1)
                # Y = X*X for the whole slab, column-split across engines
                yy = yyp.tile([P, CH, D], fp16, tag="yy")
                nc.vector.tensor_tensor(
                    out=yy[:, :L, :Y_WV], in0=fx[:, :L, :Y_WV],
                    in1=fx[:, :L, :Y_WV], op=Alu.mult)
                nc.scalar.activation(yy[:, :L, Y_WV:], fx[:, :L, Y_WV:],
                                     Act.Square)
                for j in range(L):
                    ti = t + j
                    c = 0 if ti < T0 else 1
                    nc.tensor.matmul(out=ps_ssq[c][:], lhsT=ohs[j][:],
                                     rhs=yy[:, j, :],
                                     start=ti == 0 or ti == T0,
                                     stop=ti == T0 - 1 or ti == T - 1)
                if t < T0 <= t + L:  # chunk 0 just finished accumulating
                    readout(0)
                t += L

            readout(1)
            nc.sync.dma_start(out=stats, in_=out_t[:])

    nc.compile()
    return nc


def _shard(feats, labels, demog):
    """Partition rows by (demog, label-half) across cores, then bucket by
    group-chunk (local label < 128 vs >= 128) within each core.

    Any row order within a bucket works: the device one-hot (iota vs
    label mod 128) routes each row to its group slot. Also returns the
    exact per-(core, chunk, slot) counts -- sharding metadata the host
    keeps for the finale.
    """
    half = (labels >= NL).astype(np.int32)
    shard_id = demog * 2 + half
    local = labels % NL          # 0..255 within the core
    chunk = local // P           # 0 or 1
    slot = (local % P).astype(np.float32)

    cnt = np.zeros((N_CORES, 2, P), np.int64)
    np.add.at(cnt, (shard_id, chunk, local % P), 1)

    feats16 = feats.astype(np.float16)
    rows0, rows1 = [], []
    for s in range(N_CORES):
        m = shard_id == s
        rows0.append(np.flatnonzero(m & (chunk == 0)))
        rows1.append(np.flatnonzero(m & (chunk == 1)))
    T0 = max(1, max(-(-len(r) // P) for r in rows0))
    T1 = max(1, max(-(-len(r) // P) for r in rows1))
    T = T0 + T1

    in_maps = []
    for s in range(N_CORES):
        f = np.zeros((T * P, D), np.float16)
        lab = np.full(T * P, PAD_LAB, np.float32)
        r0, r1 = rows0[s], rows1[s]
        f[:len(r0)] = feats16[r0]
        lab[:len(r0)] = slot[r0]
        f[T0 * P:T0 * P + len(r1)] = feats16[r1]
        lab[T0 * P:T0 * P + len(r1)] = slot[r1]
        # tile t, partition p <-> row t*128+p; device reads [P, T, D]
        ft = np.ascontiguousarray(f.reshape(T, P, D).transpose(1, 0, 2))
        lt = np.ascontiguousarray(lab.reshape(T, P).T)
        in_maps.append({"feats_t": ft, "labels_t": lt})
    return T0, T1, in_maps, cnt


def _combine(stats_per_core, cnt):
    """Host finale over the 2048 groups (the 'gather/unshard' step)."""
    num = np.zeros(ND, np.float64)
    den = np.zeros(ND, np.float64)
    for s, st in enumerate(stats_per_core):
        st = st.astype(np.float64)
        d = s // 2
        for c in range(2):
            norm2 = st[:, c]
            sumsq = st[:, 2 + c]
            cg = cnt[s, c].astype(np.float64)
            safe = np.maximum(cg, 1.0)
            grp = (sumsq - norm2 / safe) / safe
            pres = cg > 0
            num[d] += grp[pres].sum()
            den[d] += pres.sum()
    intra = num / np.maximum(den, 1.0)
    return np.float32(np.mean(np.abs(intra - intra.mean())))


def kernel(feats, labels, demog_labels, _results_out=None):
    feats = np.ascontiguousarray(np.asarray(feats), dtype=np.float32)
    labels = np.asarray(labels).astype(np.int32)
    demog = np.asarray(demog_labels).astype(np.int32)
    assert feats.ndim == 2 and feats.shape[1] == D

    T0, T1, in_maps, cnt = _shard(feats, labels, demog)
    key = (T0, T1)
    nc = _cache.get(key)
    if nc is None:
        nc = _cache.setdefault(key, _build(T0, T1))
    res = None
    last_exc = None
    for attempt in range(3):
        try:
            res = bass_utils.run_bass_kernel_spmd(
                nc, in_maps, core_ids=list(range(N_CORES)))
            break
        except Exception as e:  # transient axon worker hangups
            last_exc = e
            import time
            time.sleep(10)
    if res is None:
        raise last_exc
    if _results_out is not None:
        _results_out.append(res)
    return _combine([res.results[s]["stats"] for s in range(N_CORES)], cnt)
